# revision 3
# baseline (speedup 1.0000x reference)
"""EDRN cell kernel for Trainium2, data-parallel over batch across 8 NeuronCores.

Strategy:
  - Shard batch B=1024 into 8 slices of 128 rows; replicate weights (fp16).
  - Per core, mapping: batch rows on PSUM partitions, gate columns on the free
    dim.  Stationary operands are transposed activation slices (host-prepped,
    fp16), moving operands are weight row-chunks (fp16), accumulate fp32 PSUM.
  - Biases folded in as K=1 matmuls against a ones-row stationary.
  - A_pt (block-diagonal strictly-upper 4x4) handled as 16 [128,128] diagonal
    block matmuls.
  - a_new_last / aa_last stationaries produced on device via fp16 copy + DMA
    transpose.
All elementwise math is fp32; only matmul operands are fp16 (measured end to
end error ~2e-4 relative).
"""

import numpy as np

import concourse.bass as bass
import concourse.mybir as mybir
import concourse.tile as tile
from concourse import bacc
from concourse.bass_utils import run_bass_kernel_spmd

N, M, D = 256, 512, 4
MD = M * D  # 2048
B = 1024
NCORES = 8
BL = B // NCORES  # 128 batch rows per core

F16 = mybir.dt.float16
F32 = mybir.dt.float32

KM = M // 128   # 4 K-chunks for the M-sized contraction
KX = N // 128   # 2 K-chunks for the N-sized contraction
NCH = MD // 512  # 4 column chunks of 512

_CACHE = {}
LAST_RESULT = None  # BassKernelResults of the most recent run (for test harness)


def _build():
    nc = bacc.Bacc(
        "TRN2", target_bir_lowering=False, debug=False, num_devices=NCORES
    )

    def din(name, shape, dt):
        return nc.dram_tensor(name, shape, dt, kind="ExternalInput").ap()

    def dout(name, shape, dt):
        return nc.dram_tensor(name, shape, dt, kind="ExternalOutput").ap()

    mT = din("mT16", [M, BL], F16)
    xT = din("xT16", [N + 1, BL], F16)          # [x^T ; ones-row]
    aT = din("aT16", [MD, BL], F16)
    alastT = din("alastT16", [M, BL], F16)
    a32 = din("a32", [BL, MD], F32)
    Wm = {g: din(f"Wm_{g}", [M, MD], F16) for g in ("fg", "in", "th", "ot")}
    Wx = {g: din(f"Wx_{g}", [N + 1, MD], F16) for g in ("fg", "in", "th", "ot")}
    Aptd = din("Aptd16", [16, 128, 128], F16)
    Ast = din("Ast16", [M, M], F16)
    m_out = dout("m_new_out", [BL, M], F32)
    a_out = dout("a_new_out", [BL, MD], F32)

    AF = mybir.ActivationFunctionType
    OP = mybir.AluOpType

    with tile.TileContext(nc) as tc:
        with (
            tc.tile_pool(name="singles", bufs=1) as singles,
            tc.tile_pool(name="wpool", bufs=8) as wpool,
            tc.tile_pool(name="psum", bufs=8, space="PSUM") as pp,
            tc.tile_pool(name="work", bufs=3) as work,
        ):
            # ---- resident stationaries / constants ----
            smT = singles.tile([128, KM, 128], F16, tag="smT")
            nc.sync.dma_start(out=smT, in_=mT.rearrange("(c p) b -> p c b", p=128))
            sxT = singles.tile([128, KX, 128], F16, tag="sxT")
            nc.sync.dma_start(
                out=sxT, in_=xT[0:N].rearrange("(c p) b -> p c b", p=128)
            )
            sones = singles.tile([1, 128], F16, tag="sones")
            nc.sync.dma_start(out=sones, in_=xT[N : N + 1])
            saT = singles.tile([128, 16, 128], F16, tag="saT")
            nc.sync.dma_start(out=saT, in_=aT.rearrange("(c p) b -> p c b", p=128))
            salastT = singles.tile([128, KM, 128], F16, tag="salastT")
            nc.sync.dma_start(
                out=salastT, in_=alastT.rearrange("(c p) b -> p c b", p=128)
            )
            sAptd = singles.tile([128, 16, 128], F16, tag="sAptd")
            nc.sync.dma_start(out=sAptd, in_=Aptd.rearrange("c p k -> p c k"))
            sAst = singles.tile([128, KM, 512], F16, tag="sAst")
            nc.sync.dma_start(
                out=sAst, in_=Ast.rearrange("(c p) m -> p c m", p=128)
            )
            sa32 = singles.tile([128, MD], F32, tag="sa32")
            nc.sync.dma_start(out=sa32, in_=a32)

            def gate(gname, statA, func, with_pt):
                """Compute G = func(statA.T @ Wm_g + [x|1].T @ Wx_g (+ pt))."""
                G = singles.tile([128, MD], F32, tag=f"G_{gname}")
                psums = [pp.tile([128, 512], F32, tag="ps", name=f"ps_{gname}_{n}") for n in range(NCH)]
                for k in range(KM):
                    w = wpool.tile([128, MD], F16, tag="w")
                    nc.sync.dma_start(
                        out=w, in_=Wm[gname][128 * k : 128 * (k + 1), :]
                    )
                    for n in range(NCH):
                        nc.tensor.matmul(
                            psums[n],
                            lhsT=statA[:, k, :],
                            rhs=w[:, 512 * n : 512 * (n + 1)],
                            start=(k == 0),
                            stop=False,
                        )
                for k in range(KX):
                    w = wpool.tile([128, MD], F16, tag="w")
                    nc.sync.dma_start(
                        out=w, in_=Wx[gname][128 * k : 128 * (k + 1), :]
                    )
                    for n in range(NCH):
                        nc.tensor.matmul(
                            psums[n],
                            lhsT=sxT[:, k, :],
                            rhs=w[:, 512 * n : 512 * (n + 1)],
                            start=False,
                            stop=False,
                        )
                if with_pt:
                    for n in range(NCH):
                        for s in range(4):
                            c = 4 * n + s
                            nc.tensor.matmul(
                                psums[n][:, 128 * s : 128 * (s + 1)],
                                lhsT=saT[:, c, :],
                                rhs=sAptd[:, c, :],
                                start=False,
                                stop=False,
                                skip_group_check=True,
                            )
                wb = wpool.tile([1, MD], F16, tag="wb")
                nc.sync.dma_start(out=wb, in_=Wx[gname][N : N + 1, :])
                for n in range(NCH):
                    nc.tensor.matmul(
                        psums[n],
                        lhsT=sones,
                        rhs=wb[:, 512 * n : 512 * (n + 1)],
                        start=False,
                        stop=True,
                    )
                for n in range(NCH):
                    nc.scalar.activation(
                        out=G[:, 512 * n : 512 * (n + 1)], in_=psums[n], func=func
                    )
                return G

            G_fg = gate("fg", smT, AF.Sigmoid, False)
            G_in = gate("in", smT, AF.Sigmoid, False)
            G_th = gate("th", salastT, AF.Tanh, True)

            # ---- a_new = a * G_fg + G_th * G_in, plus last-slice transpose ----
            a_new = singles.tile([128, MD], F32, tag="a_new")
            anl16 = singles.tile([128, 512], F16, tag="anl16")
            sanlT = singles.tile([128, KM, 128], F16, tag="sanlT")
            for n in range(NCH):
                sl = slice(512 * n, 512 * (n + 1))
                t1 = work.tile([128, 512], F32, tag="t1")
                nc.vector.tensor_mul(t1, G_th[:, sl], G_in[:, sl])
                t2 = work.tile([128, 512], F32, tag="t2")
                nc.vector.tensor_mul(t2, sa32[:, sl], G_fg[:, sl])
                nc.vector.tensor_add(a_new[:, sl], t1, t2)
                # last-of-4 columns of this chunk -> fp16, then transpose
                lastview = a_new[:, sl].rearrange("p (m s) -> p m s", s=4)[:, :, 3]
                nc.vector.tensor_copy(anl16[:, 128 * n : 128 * (n + 1)], lastview)
                nc.sync.dma_start(
                    out=sanlT[:, n, :],
                    in_=anl16[:, 128 * n : 128 * (n + 1)],
                    transpose=True,
                )
            nc.sync.dma_start(out=a_out, in_=a_new)

            G_ot = gate("ot", sanlT, AF.Sigmoid, False)

            # ---- aa = tanh(a_new) * G_ot; aa_last transpose ----
            aa = singles.tile([128, MD], F32, tag="aa")
            aal16 = singles.tile([128, 512], F16, tag="aal16")
            saalT = singles.tile([128, KM, 128], F16, tag="saalT")
            for n in range(NCH):
                sl = slice(512 * n, 512 * (n + 1))
                th = work.tile([128, 512], F32, tag="th")
                nc.scalar.activation(out=th, in_=a_new[:, sl], func=AF.Tanh)
                nc.vector.tensor_mul(aa[:, sl], th, G_ot[:, sl])
                lastview = aa[:, sl].rearrange("p (m s) -> p m s", s=4)[:, :, 3]
                nc.vector.tensor_copy(aal16[:, 128 * n : 128 * (n + 1)], lastview)
                nc.sync.dma_start(
                    out=saalT[:, n, :],
                    in_=aal16[:, 128 * n : 128 * (n + 1)],
                    transpose=True,
                )

            # ---- m_new = sum_{s<3} aa[:, :, s] + aa_last @ A_st ----
            psm = pp.tile([128, 512], F32, tag="ps")
            for k in range(KM):
                nc.tensor.matmul(
                    psm,
                    lhsT=saalT[:, k, :],
                    rhs=sAst[:, k, :],
                    start=(k == 0),
                    stop=(k == KM - 1),
                )
            aav = aa.rearrange("p (m s) -> p m s", s=4)
            s01 = work.tile([128, 512], F32, tag="s01")
            nc.vector.tensor_add(s01, aav[:, :, 0], aav[:, :, 1])
            s012 = work.tile([128, 512], F32, tag="s012")
            nc.vector.tensor_add(s012, s01, aav[:, :, 2])
            m_new = singles.tile([128, 512], F32, tag="m_new")
            nc.vector.tensor_add(m_new, s012, psm)
            nc.sync.dma_start(out=m_out, in_=m_new)

    nc.compile()
    return nc


def _get_nc():
    if "nc" not in _CACHE:
        _CACHE["nc"] = _build()
    return _CACHE["nc"]


def _prep_inputs(inputs):
    f16 = np.float16
    f32 = np.float32
    x_t = np.asarray(inputs["x_t"], f32)
    m_t = np.asarray(inputs["m_t"], f32)
    a_t = np.asarray(inputs["a_t"], f32)

    # masks (idempotent with how setup_inputs builds the weights)
    eye = np.eye(M, dtype=f32)
    diag_mask = np.broadcast_to((1.0 - eye)[:, :, None], (M, M, D)).reshape(M, MD)
    A_th = np.asarray(inputs["A_th"], f32) * diag_mask
    A_ot = np.asarray(inputs["A_ot"], f32) * diag_mask
    tri = (np.arange(D)[:, None] < np.arange(D)[None, :]).astype(f32)
    pt_mask = (eye[:, None, :, None] * tri[None, :, None, :]).reshape(MD, MD)
    A_pt = np.asarray(inputs["A_pt"], f32) * pt_mask

    shared = {
        "Wm_fg": np.asarray(inputs["A_fg"], f32).astype(f16),
        "Wm_in": np.asarray(inputs["A_in"], f32).astype(f16),
        "Wm_th": A_th.astype(f16),
        "Wm_ot": A_ot.astype(f16),
        "Ast16": np.asarray(inputs["A_st"], f32).astype(f16),
        "Aptd16": np.ascontiguousarray(
            np.stack(
                [A_pt[128 * c : 128 * (c + 1), 128 * c : 128 * (c + 1)] for c in range(16)]
            )
        ).astype(f16),
    }
    for g in ("fg", "in", "th", "ot"):
        shared[f"Wx_{g}"] = np.ascontiguousarray(
            np.concatenate(
                [np.asarray(inputs[f"B_{g}"], f32), np.asarray(inputs[f"b_{g}"], f32)],
                axis=0,
            )
        ).astype(f16)

    ones_row = np.ones((1, BL), f16)
    in_maps = []
    for i in range(NCORES):
        sl = slice(BL * i, BL * (i + 1))
        xs, ms, as_ = x_t[sl], m_t[sl], a_t[sl]
        im = dict(shared)
        im["mT16"] = np.ascontiguousarray(ms.T).astype(f16)
        im["xT16"] = np.concatenate(
            [np.ascontiguousarray(xs.T).astype(f16), ones_row], axis=0
        )
        im["aT16"] = np.ascontiguousarray(as_.T).astype(f16)
        im["alastT16"] = np.ascontiguousarray(as_[:, 3::4].T).astype(f16)
        im["a32"] = np.ascontiguousarray(as_)
        in_maps.append(im)
    return in_maps


def kernel(**inputs):
    global LAST_RESULT
    nc = _get_nc()
    in_maps = _prep_inputs(inputs)
    res = run_bass_kernel_spmd(nc, in_maps, list(range(NCORES)))
    LAST_RESULT = res
    m_new = np.concatenate([res.results[i]["m_new_out"] for i in range(NCORES)], axis=0)
    a_new = np.concatenate([res.results[i]["a_new_out"] for i in range(NCORES)], axis=0)
    return (m_new, a_new)


# revision 4
# speedup vs baseline: 1.0151x; 1.0151x over previous
"""EDRN cell kernel for Trainium2, data-parallel over batch across 8 NeuronCores.

Strategy:
  - Shard batch B=1024 into 8 slices of 128 rows; replicate weights (fp16).
  - Per core, mapping: batch rows on PSUM partitions, gate columns on the free
    dim.  Stationary operands are transposed activation slices (host-prepped,
    fp16, pre-laid-out as [128, c, 128] so every DMA is contiguous per
    partition), moving operands are weight row-chunks (fp16), fp32 PSUM.
  - Per gate the A-part and B-part weights are merged host-side into one
    [128, 6, 2048] array (768 rows = 512 m-rows + 256 x-rows); the bias is a
    K=1 matmul against a ones-row stationary.
  - A_pt (block-diagonal strictly-upper 4x4) handled as 16 [128,128] diagonal
    block matmuls.
  - a_new_last / aa_last stationaries produced on device via fp16 copy + DMA
    transpose.
Elementwise math is fp32; matmul operands and the streamed copy of `a` are
fp16 (measured end-to-end error ~4e-4 relative).
"""

import numpy as np

import concourse.bass as bass
import concourse.mybir as mybir
import concourse.tile as tile
from concourse import bacc
from concourse.bass_utils import run_bass_kernel_spmd

N, M, D = 256, 512, 4
MD = M * D  # 2048
B = 1024
NCORES = 8
BL = B // NCORES  # 128 batch rows per core

F16 = mybir.dt.float16
F32 = mybir.dt.float32

KM = M // 128    # 4 K-chunks of the m/a_last contraction
KX = N // 128    # 2 K-chunks of the x contraction
KW = KM + KX     # 6 merged weight K-chunks per gate
NCH = MD // 512  # 4 column chunks of 512

_CACHE = {}
LAST_RESULT = None  # BassKernelResults of the most recent run (for test harness)


def _build():
    nc = bacc.Bacc(
        "TRN2", target_bir_lowering=False, debug=False, num_devices=NCORES
    )

    def din(name, shape, dt):
        return nc.dram_tensor(name, shape, dt, kind="ExternalInput").ap()

    def dout(name, shape, dt):
        return nc.dram_tensor(name, shape, dt, kind="ExternalOutput").ap()

    # stationaries, host-prepped to [128, chunks, 128] contiguous layouts
    mxT = din("mxT16", [128, KW, 128], F16)    # m^T chunks 0..3, x^T chunks 4..5
    ones = din("ones16", [1, 128], F16)
    aT = din("aT16", [128, 16, 128], F16)      # a^T chunks (pt stationary)
    alastT = din("alastT16", [128, KM, 128], F16)
    a16 = din("a16", [BL, MD], F16)
    # merged weights per gate: [A_g ; B_g] rows as [128, 6, 2048], + bias row
    Wg = {g: din(f"W_{g}", [128, KW, MD], F16) for g in ("fg", "in", "th", "ot")}
    Wb = {g: din(f"Wb_{g}", [1, MD], F16) for g in ("fg", "in", "th", "ot")}
    Aptd = din("Aptd16", [128, 16, 128], F16)  # diag blocks, pre-transposed layout
    Ast = din("Ast16", [128, KM, 512], F16)
    m_out = dout("m_new_out", [BL, M], F32)
    a_out = dout("a_new_out", [BL, MD], F32)

    AF = mybir.ActivationFunctionType

    with tile.TileContext(nc) as tc:
        with (
            tc.tile_pool(name="singles", bufs=1) as singles,
            tc.tile_pool(name="wpool", bufs=8) as wpool,
            tc.tile_pool(name="psum", bufs=8, space="PSUM") as pp,
            tc.tile_pool(name="work", bufs=3) as work,
        ):
            # tiny stationaries first so the first gate can start immediately
            smxT = singles.tile([128, KW, 128], F16, tag="smxT")
            nc.gpsimd.dma_start(out=smxT, in_=mxT)
            sones = singles.tile([1, 128], F16, tag="sones")
            nc.gpsimd.dma_start(out=sones, in_=ones)

            def gate(gname, statA, func, with_pt, saT=None, sAptd=None):
                G = singles.tile([128, MD], F32, tag=f"G_{gname}")
                psums = [
                    pp.tile([128, 512], F32, tag="ps", name=f"ps_{gname}_{n}")
                    for n in range(NCH)
                ]
                for k in range(KW):
                    w = wpool.tile([128, MD], F16, tag="w")
                    nc.sync.dma_start(out=w, in_=Wg[gname][:, k, :])
                    stat = statA[:, k, :] if k < KM else smxT[:, k, :]
                    for n in range(NCH):
                        nc.tensor.matmul(
                            psums[n],
                            lhsT=stat,
                            rhs=w[:, 512 * n : 512 * (n + 1)],
                            start=(k == 0),
                            stop=False,
                        )
                if with_pt:
                    for n in range(NCH):
                        for s in range(4):
                            c = 4 * n + s
                            nc.tensor.matmul(
                                psums[n][:, 128 * s : 128 * (s + 1)],
                                lhsT=saT[:, c, :],
                                rhs=sAptd[:, c, :],
                                start=False,
                                stop=False,
                                skip_group_check=True,
                            )
                wb = wpool.tile([1, MD], F16, tag="wb")
                nc.sync.dma_start(out=wb, in_=Wb[gname])
                for n in range(NCH):
                    nc.tensor.matmul(
                        psums[n],
                        lhsT=sones,
                        rhs=wb[:, 512 * n : 512 * (n + 1)],
                        start=False,
                        stop=True,
                    )
                for n in range(NCH):
                    nc.scalar.activation(
                        out=G[:, 512 * n : 512 * (n + 1)], in_=psums[n], func=func
                    )
                return G

            G_fg = gate("fg", smxT, AF.Sigmoid, False)

            # stationaries needed by the th gate — DMA while fg/in compute
            salastT = singles.tile([128, KM, 128], F16, tag="salastT")
            nc.gpsimd.dma_start(out=salastT, in_=alastT)
            saT = singles.tile([128, 16, 128], F16, tag="saT")
            nc.gpsimd.dma_start(out=saT, in_=aT)
            sAptd = singles.tile([128, 16, 128], F16, tag="sAptd")
            nc.gpsimd.dma_start(out=sAptd, in_=Aptd)
            sa16 = singles.tile([128, MD], F16, tag="sa16")
            nc.gpsimd.dma_start(out=sa16, in_=a16)

            G_in = gate("in", smxT, AF.Sigmoid, False)
            G_th = gate("th", salastT, AF.Tanh, True, saT=saT, sAptd=sAptd)

            sAst = singles.tile([128, KM, 512], F16, tag="sAst")
            nc.gpsimd.dma_start(out=sAst, in_=Ast)

            # ---- a_new = a * G_fg + G_th * G_in, plus last-slice transpose ----
            a_new = singles.tile([128, MD], F32, tag="a_new")
            anl16 = singles.tile([128, 512], F16, tag="anl16")
            sanlT = singles.tile([128, KM, 128], F16, tag="sanlT")
            for n in range(NCH):
                sl = slice(512 * n, 512 * (n + 1))
                t1 = work.tile([128, 512], F32, tag="t1")
                nc.vector.tensor_mul(t1, G_th[:, sl], G_in[:, sl])
                t2 = work.tile([128, 512], F32, tag="t2")
                nc.vector.tensor_mul(t2, sa16[:, sl], G_fg[:, sl])
                nc.vector.tensor_add(a_new[:, sl], t1, t2)
                lastview = a_new[:, sl].rearrange("p (m s) -> p m s", s=4)[:, :, 3]
                nc.vector.tensor_copy(anl16[:, 128 * n : 128 * (n + 1)], lastview)
                nc.sync.dma_start(
                    out=sanlT[:, n, :],
                    in_=anl16[:, 128 * n : 128 * (n + 1)],
                    transpose=True,
                )
            nc.gpsimd.dma_start(out=a_out, in_=a_new)

            G_ot = gate("ot", sanlT, AF.Sigmoid, False)

            # ---- aa = tanh(a_new) * G_ot; aa_last transpose ----
            aa = singles.tile([128, MD], F32, tag="aa")
            aal16 = singles.tile([128, 512], F16, tag="aal16")
            saalT = singles.tile([128, KM, 128], F16, tag="saalT")
            for n in range(NCH):
                sl = slice(512 * n, 512 * (n + 1))
                th = work.tile([128, 512], F32, tag="th")
                nc.scalar.activation(out=th, in_=a_new[:, sl], func=AF.Tanh)
                nc.vector.tensor_mul(aa[:, sl], th, G_ot[:, sl])
                lastview = aa[:, sl].rearrange("p (m s) -> p m s", s=4)[:, :, 3]
                nc.vector.tensor_copy(aal16[:, 128 * n : 128 * (n + 1)], lastview)
                nc.sync.dma_start(
                    out=saalT[:, n, :],
                    in_=aal16[:, 128 * n : 128 * (n + 1)],
                    transpose=True,
                )

            # ---- m_new = sum_{s<3} aa[:, :, s] + aa_last @ A_st ----
            psm = pp.tile([128, 512], F32, tag="ps")
            for k in range(KM):
                nc.tensor.matmul(
                    psm,
                    lhsT=saalT[:, k, :],
                    rhs=sAst[:, k, :],
                    start=(k == 0),
                    stop=(k == KM - 1),
                )
            aav = aa.rearrange("p (m s) -> p m s", s=4)
            s01 = work.tile([128, 512], F32, tag="s01")
            nc.vector.tensor_add(s01, aav[:, :, 0], aav[:, :, 1])
            s012 = work.tile([128, 512], F32, tag="s012")
            nc.vector.tensor_add(s012, s01, aav[:, :, 2])
            m_new = singles.tile([128, 512], F32, tag="m_new")
            nc.vector.tensor_add(m_new, s012, psm)
            nc.gpsimd.dma_start(out=m_out, in_=m_new)

    nc.compile()
    return nc


def _get_nc():
    if "nc" not in _CACHE:
        _CACHE["nc"] = _build()
    return _CACHE["nc"]


def _chunked_T(x, nchunks):
    """[rows, cols] -> [128, nchunks, cols] with out[p, c, :] = x[c*128+p, :]."""
    rows, cols = x.shape
    assert rows == nchunks * 128
    return np.ascontiguousarray(x.reshape(nchunks, 128, cols).transpose(1, 0, 2))


def _prep_inputs(inputs):
    f16 = np.float16
    f32 = np.float32
    x_t = np.asarray(inputs["x_t"], f32)
    m_t = np.asarray(inputs["m_t"], f32)
    a_t = np.asarray(inputs["a_t"], f32)

    # masks (idempotent with how setup_inputs builds the weights)
    eye = np.eye(M, dtype=f32)
    diag_mask = np.broadcast_to((1.0 - eye)[:, :, None], (M, M, D)).reshape(M, MD)
    A_th = np.asarray(inputs["A_th"], f32) * diag_mask
    A_ot = np.asarray(inputs["A_ot"], f32) * diag_mask
    tri = (np.arange(D)[:, None] < np.arange(D)[None, :]).astype(f32)
    pt_mask = (eye[:, None, :, None] * tri[None, :, None, :]).reshape(MD, MD)
    A_pt = np.asarray(inputs["A_pt"], f32) * pt_mask

    Am = {
        "fg": np.asarray(inputs["A_fg"], f32),
        "in": np.asarray(inputs["A_in"], f32),
        "th": A_th,
        "ot": A_ot,
    }
    shared = {
        "Ast16": _chunked_T(np.asarray(inputs["A_st"], f32).astype(f16), KM),
        "Aptd16": _chunked_T(
            np.concatenate(
                [A_pt[128 * c : 128 * (c + 1), 128 * c : 128 * (c + 1)] for c in range(16)],
                axis=0,
            ).astype(f16),
            16,
        ),
        "ones16": np.ones((1, 128), f16),
    }
    for g in ("fg", "in", "th", "ot"):
        merged = np.concatenate([Am[g], np.asarray(inputs[f"B_{g}"], f32)], axis=0)
        shared[f"W_{g}"] = _chunked_T(merged.astype(f16), KW)
        shared[f"Wb_{g}"] = np.asarray(inputs[f"b_{g}"], f32).astype(f16).reshape(1, MD)

    in_maps = []
    for i in range(NCORES):
        sl = slice(BL * i, BL * (i + 1))
        xs, ms, as_ = x_t[sl], m_t[sl], a_t[sl]
        im = dict(shared)
        mxT = np.concatenate(
            [np.ascontiguousarray(ms.T), np.ascontiguousarray(xs.T)], axis=0
        ).astype(f16)  # [768, 128]
        im["mxT16"] = _chunked_T(mxT, KW)
        im["aT16"] = _chunked_T(np.ascontiguousarray(as_.T).astype(f16), 16)
        im["alastT16"] = _chunked_T(
            np.ascontiguousarray(as_[:, 3::4].T).astype(f16), KM
        )
        im["a16"] = as_.astype(f16)
        in_maps.append(im)
    return in_maps


def kernel(**inputs):
    global LAST_RESULT
    nc = _get_nc()
    in_maps = _prep_inputs(inputs)
    res = run_bass_kernel_spmd(nc, in_maps, list(range(NCORES)))
    LAST_RESULT = res
    m_new = np.concatenate([res.results[i]["m_new_out"] for i in range(NCORES)], axis=0)
    a_new = np.concatenate([res.results[i]["a_new_out"] for i in range(NCORES)], axis=0)
    return (m_new, a_new)


# revision 5
# speedup vs baseline: 1.1310x; 1.1142x over previous
"""EDRN cell kernel for Trainium2, data-parallel over batch across 8 NeuronCores.

Strategy:
  - Shard batch B=1024 into 8 slices of 128 rows; replicate weights (fp16).
  - Mapping: batch rows on PSUM partitions, gate columns on the free dim.
    Stationary operands are transposed activation slices (host-prepped fp16,
    laid out [128, c, 128] so DMAs are contiguous), moving operands are weight
    row-chunks (fp16), fp32 PSUM accumulation.
  - Per gate the A-part and B-part weights are merged host-side into one
    [128, 6, 2048] array (768 rows = 512 m-rows + 256 x-rows), streamed as 3
    ascending-size DMAs; biases are K=1 matmuls against a ones-row stationary.
  - A_pt (block-diagonal strictly-upper 4x4) -> 16 [128,128] diag-block
    matmuls; all remaining stationaries arrive in ONE merged 2.3MB DMA.
  - a_new_last / aa_last transposed on the PE (transpose mode + identity),
    keeping the DMA queues free.
Elementwise math is fp32; matmul operands and the streamed copy of `a` are
fp16 (measured end-to-end error ~4e-4 relative).
"""

import numpy as np

import concourse.bass as bass
import concourse.mybir as mybir
import concourse.tile as tile
from concourse import bacc
from concourse.bass_utils import run_bass_kernel_spmd

N, M, D = 256, 512, 4
MD = M * D  # 2048
B = 1024
NCORES = 8
BL = B // NCORES  # 128 batch rows per core

F16 = mybir.dt.float16
F32 = mybir.dt.float32

KM = M // 128    # 4 K-chunks of the m/a_last contraction
KX = N // 128    # 2 K-chunks of the x contraction
KW = KM + KX     # 6 merged weight K-chunks per gate
NCH = MD // 512  # 4 column chunks of 512
SEGS = [(0, 1), (1, 2), (3, 3)]  # (start_chunk, n_chunks) weight DMA segments

# merged stationary blob offsets (in fp16 elements of the free dim)
O_ALAST = 0            # [128, 4*128]   a_last^T chunks
O_AT = 512             # [128, 16*128]  a^T chunks
O_APT = O_AT + 2048    # [128, 16*128]  A_pt diag blocks
O_AST = O_APT + 2048   # [128, 4*512]   A_st chunks
O_A16 = O_AST + 2048   # [128, 2048]    a (natural layout, fp16)
O_ID = O_A16 + 2048    # [128, 128]     identity
STAT_F = O_ID + 128    # 8832

_CACHE = {}
LAST_RESULT = None  # BassKernelResults of the most recent run (for test harness)


def _build():
    nc = bacc.Bacc(
        "TRN2", target_bir_lowering=False, debug=False, num_devices=NCORES
    )

    def din(name, shape, dt):
        return nc.dram_tensor(name, shape, dt, kind="ExternalInput").ap()

    def dout(name, shape, dt):
        return nc.dram_tensor(name, shape, dt, kind="ExternalOutput").ap()

    mxT = din("mxT16", [128, KW, 128], F16)   # m^T chunks 0..3, x^T chunks 4..5
    stat = din("stat16", [128, STAT_F], F16)  # merged stationaries
    wball = din("wball16", [1, 4 * MD + 128], F16)  # 4 bias rows + ones row
    Wg = {g: din(f"W_{g}", [128, KW, MD], F16) for g in ("fg", "in", "th", "ot")}
    m_out = dout("m_new_out", [BL, M], F32)
    a_out = dout("a_new_out", [BL, MD], F32)

    GIDX = {"fg": 0, "in": 1, "th": 2, "ot": 3}
    AF = mybir.ActivationFunctionType

    with tile.TileContext(nc) as tc:
        with (
            tc.tile_pool(name="singles", bufs=1) as singles,
            tc.tile_pool(name="wpool", bufs=2) as wpool,
            tc.tile_pool(name="psum", bufs=8, space="PSUM") as pp,
            tc.tile_pool(name="work", bufs=3) as work,
        ):
            # tiny stationaries first so the first gate can start immediately
            smxT = singles.tile([128, KW, 128], F16, tag="smxT")
            nc.gpsimd.dma_start(out=smxT, in_=mxT)
            swb = singles.tile([1, 4 * MD + 128], F16, tag="swb")
            nc.gpsimd.dma_start(out=swb, in_=wball)
            sstat = singles.tile([128, STAT_F], F16, tag="sstat")
            nc.gpsimd.dma_start(out=sstat, in_=stat)

            sones = swb[:, 4 * MD : 4 * MD + 128]
            salastT = sstat[:, O_ALAST : O_ALAST + 512].rearrange(
                "p (c k) -> p c k", k=128
            )
            saT = sstat[:, O_AT : O_AT + 2048].rearrange("p (c k) -> p c k", k=128)
            sAptd = sstat[:, O_APT : O_APT + 2048].rearrange(
                "p (c k) -> p c k", k=128
            )
            sAst = sstat[:, O_AST : O_AST + 2048].rearrange(
                "p (c m) -> p c m", m=512
            )
            sa16 = sstat[:, O_A16 : O_A16 + 2048]
            sident = sstat[:, O_ID : O_ID + 128]

            def gate(gname, statA, func, with_pt):
                G = singles.tile([128, MD], F32, tag=f"G_{gname}")
                psums = [
                    pp.tile([128, 512], F32, tag="ps", name=f"ps_{gname}_{n}")
                    for n in range(NCH)
                ]
                for start_c, nch in SEGS:
                    w = wpool.tile(
                        [128, nch, MD], F16, tag=f"w{nch}", name=f"w_{gname}_{start_c}"
                    )
                    nc.sync.dma_start(
                        out=w, in_=Wg[gname][:, start_c : start_c + nch, :]
                    )
                    for kk in range(nch):
                        k = start_c + kk
                        lhsT = statA[:, k, :] if k < KM else smxT[:, k, :]
                        for n in range(NCH):
                            nc.tensor.matmul(
                                psums[n],
                                lhsT=lhsT,
                                rhs=w[:, kk, 512 * n : 512 * (n + 1)],
                                start=(k == 0),
                                stop=False,
                            )
                if with_pt:
                    for n in range(NCH):
                        for s in range(4):
                            c = 4 * n + s
                            nc.tensor.matmul(
                                psums[n][:, 128 * s : 128 * (s + 1)],
                                lhsT=saT[:, c, :],
                                rhs=sAptd[:, c, :],
                                start=False,
                                stop=False,
                                skip_group_check=True,
                            )
                boff = GIDX[gname] * MD
                for n in range(NCH):
                    nc.tensor.matmul(
                        psums[n],
                        lhsT=sones,
                        rhs=swb[:, boff + 512 * n : boff + 512 * (n + 1)],
                        start=False,
                        stop=True,
                    )
                for n in range(NCH):
                    nc.scalar.activation(
                        out=G[:, 512 * n : 512 * (n + 1)], in_=psums[n], func=func
                    )
                return G

            def transpose128(src16, dst, n):
                """dst[:, n, :] = src16[:, 128n:128(n+1)].T via PE transpose."""
                pt = pp.tile([128, 128], F16, tag="ps", name=f"pt_{dst.name}_{n}")
                nc.tensor.transpose(
                    pt, src16[:, 128 * n : 128 * (n + 1)], sident
                )
                nc.vector.tensor_copy(dst[:, n, :], pt)

            G_fg = gate("fg", smxT, AF.Sigmoid, False)
            G_in = gate("in", smxT, AF.Sigmoid, False)
            G_th = gate("th", salastT, AF.Tanh, True)

            # ---- a_new = a * G_fg + G_th * G_in, plus last-slice transpose ----
            a_new = singles.tile([128, MD], F32, tag="a_new")
            anl16 = singles.tile([128, 512], F16, tag="anl16")
            sanlT = singles.tile([128, KM, 128], F16, tag="sanlT")
            for n in range(NCH):
                sl = slice(512 * n, 512 * (n + 1))
                t1 = work.tile([128, 512], F32, tag="t1")
                nc.vector.tensor_mul(t1, G_th[:, sl], G_in[:, sl])
                t2 = work.tile([128, 512], F32, tag="t2")
                nc.vector.tensor_mul(t2, sa16[:, sl], G_fg[:, sl])
                nc.vector.tensor_add(a_new[:, sl], t1, t2)
                lastview = a_new[:, sl].rearrange("p (m s) -> p m s", s=4)[:, :, 3]
                nc.vector.tensor_copy(anl16[:, 128 * n : 128 * (n + 1)], lastview)
                transpose128(anl16, sanlT, n)
            nc.gpsimd.dma_start(out=a_out, in_=a_new)

            G_ot = gate("ot", sanlT, AF.Sigmoid, False)

            # ---- aa = tanh(a_new) * G_ot; aa_last transpose ----
            aa = singles.tile([128, MD], F32, tag="aa")
            aal16 = singles.tile([128, 512], F16, tag="aal16")
            saalT = singles.tile([128, KM, 128], F16, tag="saalT")
            for n in range(NCH):
                sl = slice(512 * n, 512 * (n + 1))
                th = work.tile([128, 512], F32, tag="th")
                nc.scalar.activation(out=th, in_=a_new[:, sl], func=AF.Tanh)
                nc.vector.tensor_mul(aa[:, sl], th, G_ot[:, sl])
                lastview = aa[:, sl].rearrange("p (m s) -> p m s", s=4)[:, :, 3]
                nc.vector.tensor_copy(aal16[:, 128 * n : 128 * (n + 1)], lastview)
                transpose128(aal16, saalT, n)

            # ---- m_new = sum_{s<3} aa[:, :, s] + aa_last @ A_st ----
            psm = pp.tile([128, 512], F32, tag="ps")
            for k in range(KM):
                nc.tensor.matmul(
                    psm,
                    lhsT=saalT[:, k, :],
                    rhs=sAst[:, k, :],
                    start=(k == 0),
                    stop=(k == KM - 1),
                )
            aav = aa.rearrange("p (m s) -> p m s", s=4)
            s01 = work.tile([128, 512], F32, tag="s01")
            nc.vector.tensor_add(s01, aav[:, :, 0], aav[:, :, 1])
            s012 = work.tile([128, 512], F32, tag="s012")
            nc.vector.tensor_add(s012, s01, aav[:, :, 2])
            m_new = singles.tile([128, 512], F32, tag="m_new")
            nc.vector.tensor_add(m_new, s012, psm)
            nc.gpsimd.dma_start(out=m_out, in_=m_new)

    nc.compile()
    return nc


def _get_nc():
    if "nc" not in _CACHE:
        _CACHE["nc"] = _build()
    return _CACHE["nc"]


def _chunked_T(x, nchunks):
    """[rows, cols] -> [128, nchunks*cols] with out[p, c*cols:...] = x[c*128+p, :]."""
    rows, cols = x.shape
    assert rows == nchunks * 128
    return np.ascontiguousarray(
        x.reshape(nchunks, 128, cols).transpose(1, 0, 2)
    ).reshape(128, nchunks * cols)


def _prep_inputs(inputs):
    f16 = np.float16
    f32 = np.float32
    x_t = np.asarray(inputs["x_t"], f32)
    m_t = np.asarray(inputs["m_t"], f32)
    a_t = np.asarray(inputs["a_t"], f32)

    # masks (idempotent with how setup_inputs builds the weights)
    eye = np.eye(M, dtype=f32)
    diag_mask = np.broadcast_to((1.0 - eye)[:, :, None], (M, M, D)).reshape(M, MD)
    A_th = np.asarray(inputs["A_th"], f32) * diag_mask
    A_ot = np.asarray(inputs["A_ot"], f32) * diag_mask
    tri = (np.arange(D)[:, None] < np.arange(D)[None, :]).astype(f32)
    pt_mask = (eye[:, None, :, None] * tri[None, :, None, :]).reshape(MD, MD)
    A_pt = np.asarray(inputs["A_pt"], f32) * pt_mask

    Am = {
        "fg": np.asarray(inputs["A_fg"], f32),
        "in": np.asarray(inputs["A_in"], f32),
        "th": A_th,
        "ot": A_ot,
    }
    shared = {}
    for g in ("fg", "in", "th", "ot"):
        merged = np.concatenate([Am[g], np.asarray(inputs[f"B_{g}"], f32)], axis=0)
        shared[f"W_{g}"] = _chunked_T(merged.astype(f16), KW).reshape(128, KW, MD)
    shared["wball16"] = np.concatenate(
        [np.asarray(inputs[f"b_{g}"], f32).reshape(-1) for g in ("fg", "in", "th", "ot")]
        + [np.ones(128, f32)]
    ).astype(f16).reshape(1, -1)

    ast_c = _chunked_T(np.asarray(inputs["A_st"], f32).astype(f16), KM)
    aptd_c = _chunked_T(
        np.concatenate(
            [A_pt[128 * c : 128 * (c + 1), 128 * c : 128 * (c + 1)] for c in range(16)],
            axis=0,
        ).astype(f16),
        16,
    )
    ident = np.eye(128, dtype=f16)

    in_maps = []
    for i in range(NCORES):
        sl = slice(BL * i, BL * (i + 1))
        xs, ms, as_ = x_t[sl], m_t[sl], a_t[sl]
        im = dict(shared)
        mxT = np.concatenate(
            [np.ascontiguousarray(ms.T), np.ascontiguousarray(xs.T)], axis=0
        ).astype(f16)
        im["mxT16"] = _chunked_T(mxT, KW).reshape(128, KW, 128)
        alast_c = _chunked_T(np.ascontiguousarray(as_[:, 3::4].T).astype(f16), KM)
        at_c = _chunked_T(np.ascontiguousarray(as_.T).astype(f16), 16)
        im["stat16"] = np.concatenate(
            [alast_c, at_c, aptd_c, ast_c, as_.astype(f16), ident], axis=1
        )
        in_maps.append(im)
    return in_maps


def kernel(**inputs):
    global LAST_RESULT
    nc = _get_nc()
    in_maps = _prep_inputs(inputs)
    res = run_bass_kernel_spmd(nc, in_maps, list(range(NCORES)))
    LAST_RESULT = res
    m_new = np.concatenate([res.results[i]["m_new_out"] for i in range(NCORES)], axis=0)
    a_new = np.concatenate([res.results[i]["a_new_out"] for i in range(NCORES)], axis=0)
    return (m_new, a_new)


# revision 7
# speedup vs baseline: 1.1608x; 1.0264x over previous
"""EDRN cell kernel for Trainium2, data-parallel over batch across 8 NeuronCores.

Strategy:
  - Shard batch B=1024 into 8 slices of 128 rows; replicate weights (fp16).
  - Mapping: batch rows on PSUM partitions, gate columns on the free dim.
    Stationary operands are transposed activation slices (host-prepped fp16,
    laid out [128, c, 128] so DMAs are contiguous), moving operands are weight
    row-chunks (fp16), fp32 PSUM accumulation.
  - Per gate the B(x)-part and A-part weights are merged host-side into one
    [128, 6, 2048] array with the x-part FIRST (so the ot gate can overlap its
    x-matmuls with the a_new elementwise phase), streamed as 3 ascending-size
    DMAs; biases are K=1 matmuls against a ones-row stationary.
  - A_pt (block-diagonal strictly-upper 4x4) -> 16 [128,128] diag-block
    matmuls; other stationaries arrive in two merged DMAs timed to their use.
  - a_new_last / aa_last transposed on the PE (transpose mode + identity).
Elementwise math is fp32; matmul operands and the streamed copy of `a` are
fp16 (measured end-to-end error ~4e-4 relative).
"""

import numpy as np

import concourse.bass as bass
import concourse.mybir as mybir
import concourse.tile as tile
from concourse import bacc
from concourse.bass_utils import run_bass_kernel_spmd

N, M, D = 256, 512, 4
MD = M * D  # 2048
B = 1024
NCORES = 8
BL = B // NCORES  # 128 batch rows per core

F16 = mybir.dt.float16
F32 = mybir.dt.float32

KM = M // 128    # 4 K-chunks of the m/a_last contraction
KX = N // 128    # 2 K-chunks of the x contraction
KW = KM + KX     # 6 merged weight K-chunks per gate (x-part first)
NCH = MD // 512  # 4 column chunks of 512
SEGS = [(0, 1), (1, 2), (3, 3)]  # (start_chunk, n_chunks) weight DMA segments

# stat_a blob: a_last^T chunks | a^T chunks | A_pt diag blocks
SA_ALAST = 0
SA_AT = 512
SA_APT = SA_AT + 2048
SA_F = SA_APT + 2048  # 4608
# stat_b blob: A_st chunks | a (natural, fp16) | identity
SB_AST = 0
SB_A16 = 2048
SB_ID = SB_A16 + 2048
SB_F = SB_ID + 128  # 4224

_CACHE = {}
LAST_RESULT = None  # BassKernelResults of the most recent run (for test harness)


def _build():
    nc = bacc.Bacc(
        "TRN2", target_bir_lowering=False, debug=False, num_devices=NCORES
    )

    def din(name, shape, dt):
        return nc.dram_tensor(name, shape, dt, kind="ExternalInput").ap()

    def dout(name, shape, dt):
        return nc.dram_tensor(name, shape, dt, kind="ExternalOutput").ap()

    mxT = din("mxT16", [128, KW, 128], F16)   # x^T chunks 0..1, m^T chunks 2..5
    stat_a = din("stat_a16", [128, SA_F], F16)
    stat_b = din("stat_b16", [128, SB_F], F16)
    wball = din("wball16", [1, 4 * MD + 128], F16)  # 4 bias rows + ones row
    Wg = {g: din(f"W_{g}", [128, KW, MD], F16) for g in ("fg", "in", "th", "ot")}
    m_out = dout("m_new_out", [BL, M], F32)
    a_out = dout("a_new_out", [BL, MD], F32)

    GIDX = {"fg": 0, "in": 1, "th": 2, "ot": 3}
    AF = mybir.ActivationFunctionType

    with tile.TileContext(nc) as tc:
        with (
            tc.tile_pool(name="singles", bufs=1) as singles,
            tc.tile_pool(name="wpool", bufs=2) as wpool,
            tc.tile_pool(name="psum", bufs=8, space="PSUM") as pp,
            tc.tile_pool(name="work", bufs=3) as work,
        ):
            # small stationaries on the scalar HWDGE ring (parallel to weights)
            smxT = singles.tile([128, KW, 128], F16, tag="smxT")
            nc.scalar.dma_start(out=smxT, in_=mxT)
            swb = singles.tile([1, 4 * MD + 128], F16, tag="swb")
            nc.scalar.dma_start(out=swb, in_=wball)
            sones = swb[:, 4 * MD : 4 * MD + 128]

            def gate(gname, statA, func, with_pt, saT=None, sAptd=None):
                G = singles.tile([128, MD], F32, tag=f"G_{gname}")
                psums = [
                    pp.tile([128, 512], F32, tag="ps", name=f"ps_{gname}_{n}")
                    for n in range(NCH)
                ]
                for start_c, nch in SEGS:
                    w = wpool.tile(
                        [128, nch, MD], F16, tag=f"w{nch}", name=f"w_{gname}_{start_c}"
                    )
                    nc.sync.dma_start(
                        out=w, in_=Wg[gname][:, start_c : start_c + nch, :]
                    )
                    for kk in range(nch):
                        k = start_c + kk
                        # chunk order: x-part (0..1) then A-part (2..5)
                        if k < KX or statA is None:
                            lhsT = smxT[:, k, :]
                        else:
                            lhsT = statA[:, k - KX, :]
                        for n in range(NCH):
                            nc.tensor.matmul(
                                psums[n],
                                lhsT=lhsT,
                                rhs=w[:, kk, 512 * n : 512 * (n + 1)],
                                start=(k == 0),
                                stop=False,
                            )
                if with_pt:
                    for n in range(NCH):
                        for s in range(4):
                            c = 4 * n + s
                            nc.tensor.matmul(
                                psums[n][:, 128 * s : 128 * (s + 1)],
                                lhsT=saT[:, c, :],
                                rhs=sAptd[:, c, :],
                                start=False,
                                stop=False,
                                skip_group_check=True,
                            )
                boff = GIDX[gname] * MD
                for n in range(NCH):
                    nc.tensor.matmul(
                        psums[n],
                        lhsT=sones,
                        rhs=swb[:, boff + 512 * n : boff + 512 * (n + 1)],
                        start=False,
                        stop=True,
                    )
                for n in range(NCH):
                    nc.scalar.activation(
                        out=G[:, 512 * n : 512 * (n + 1)], in_=psums[n], func=func
                    )
                return G

            G_fg = gate("fg", None, AF.Sigmoid, False)

            # stationaries for th / a_new, loaded while fg/in stream
            ssa = singles.tile([128, SA_F], F16, tag="ssa")
            nc.gpsimd.dma_start(out=ssa, in_=stat_a)
            salastT = ssa[:, SA_ALAST : SA_ALAST + 512].rearrange(
                "p (c k) -> p c k", k=128
            )
            saT = ssa[:, SA_AT : SA_AT + 2048].rearrange("p (c k) -> p c k", k=128)
            sAptd = ssa[:, SA_APT : SA_APT + 2048].rearrange(
                "p (c k) -> p c k", k=128
            )

            G_in = gate("in", None, AF.Sigmoid, False)

            ssb = singles.tile([128, SB_F], F16, tag="ssb")
            nc.gpsimd.dma_start(out=ssb, in_=stat_b)
            sAst = ssb[:, SB_AST : SB_AST + 2048].rearrange(
                "p (c m) -> p c m", m=512
            )
            sa16 = ssb[:, SB_A16 : SB_A16 + 2048]
            sident = ssb[:, SB_ID : SB_ID + 128]

            G_th = gate("th", salastT, AF.Tanh, True, saT=saT, sAptd=sAptd)

            def transpose128(src16, dst, n):
                """dst[:, n, :] = src16[:, 128n:128(n+1)].T via PE transpose."""
                pt = pp.tile([128, 128], F16, tag="ps", name=f"pt_{dst.name}_{n}")
                nc.tensor.transpose(
                    pt, src16[:, 128 * n : 128 * (n + 1)], sident
                )
                nc.vector.tensor_copy(dst[:, n, :], pt)

            # ---- a_new = a * G_fg + G_th * G_in, plus last-slice transpose ----
            a_new = singles.tile([128, MD], F32, tag="a_new")
            anl16 = singles.tile([128, 512], F16, tag="anl16")
            sanlT = singles.tile([128, KM, 128], F16, tag="sanlT")
            for n in range(NCH):
                sl = slice(512 * n, 512 * (n + 1))
                t1 = work.tile([128, 512], F32, tag="t1")
                nc.vector.tensor_mul(t1, G_th[:, sl], G_in[:, sl])
                t2 = work.tile([128, 512], F32, tag="t2")
                nc.gpsimd.tensor_mul(t2, sa16[:, sl], G_fg[:, sl])
                nc.vector.tensor_add(a_new[:, sl], t1, t2)
                lastview = a_new[:, sl].rearrange("p (m s) -> p m s", s=4)[:, :, 3]
                nc.vector.tensor_copy(anl16[:, 128 * n : 128 * (n + 1)], lastview)
                transpose128(anl16, sanlT, n)
            nc.gpsimd.dma_start(out=a_out, in_=a_new)

            G_ot = gate("ot", sanlT, AF.Sigmoid, False)

            # ---- aa = tanh(a_new) * G_ot; m_new accumulates as chunks finish ----
            aa = singles.tile([128, MD], F32, tag="aa")
            aal16 = singles.tile([128, 512], F16, tag="aal16")
            saalT = singles.tile([128, KM, 128], F16, tag="saalT")
            psm = pp.tile([128, 512], F32, tag="ps")
            for n in range(NCH):
                sl = slice(512 * n, 512 * (n + 1))
                th = work.tile([128, 512], F32, tag="th")
                nc.scalar.activation(out=th, in_=a_new[:, sl], func=AF.Tanh)
                nc.vector.tensor_mul(aa[:, sl], th, G_ot[:, sl])
                lastview = aa[:, sl].rearrange("p (m s) -> p m s", s=4)[:, :, 3]
                nc.vector.tensor_copy(aal16[:, 128 * n : 128 * (n + 1)], lastview)
                transpose128(aal16, saalT, n)
                nc.tensor.matmul(
                    psm,
                    lhsT=saalT[:, n, :],
                    rhs=sAst[:, n, :],
                    start=(n == 0),
                    stop=(n == NCH - 1),
                )
            aav = aa.rearrange("p (m s) -> p m s", s=4)
            s01 = work.tile([128, 512], F32, tag="s01")
            nc.vector.tensor_add(s01, aav[:, :, 0], aav[:, :, 1])
            s012 = work.tile([128, 512], F32, tag="s012")
            nc.vector.tensor_add(s012, s01, aav[:, :, 2])
            m_new = singles.tile([128, 512], F32, tag="m_new")
            nc.vector.tensor_add(m_new, s012, psm)
            nc.gpsimd.dma_start(out=m_out, in_=m_new)

    nc.compile()
    return nc


def _get_nc():
    if "nc" not in _CACHE:
        _CACHE["nc"] = _build()
    return _CACHE["nc"]


def _chunked_T(x, nchunks):
    """[rows, cols] -> [128, nchunks*cols] with out[p, c*cols:...] = x[c*128+p, :]."""
    rows, cols = x.shape
    assert rows == nchunks * 128
    return np.ascontiguousarray(
        x.reshape(nchunks, 128, cols).transpose(1, 0, 2)
    ).reshape(128, nchunks * cols)


def _prep_inputs(inputs):
    f16 = np.float16
    f32 = np.float32
    x_t = np.asarray(inputs["x_t"], f32)
    m_t = np.asarray(inputs["m_t"], f32)
    a_t = np.asarray(inputs["a_t"], f32)

    # masks (idempotent with how setup_inputs builds the weights)
    eye = np.eye(M, dtype=f32)
    diag_mask = np.broadcast_to((1.0 - eye)[:, :, None], (M, M, D)).reshape(M, MD)
    A_th = np.asarray(inputs["A_th"], f32) * diag_mask
    A_ot = np.asarray(inputs["A_ot"], f32) * diag_mask
    tri = (np.arange(D)[:, None] < np.arange(D)[None, :]).astype(f32)
    pt_mask = (eye[:, None, :, None] * tri[None, :, None, :]).reshape(MD, MD)
    A_pt = np.asarray(inputs["A_pt"], f32) * pt_mask

    Am = {
        "fg": np.asarray(inputs["A_fg"], f32),
        "in": np.asarray(inputs["A_in"], f32),
        "th": A_th,
        "ot": A_ot,
    }
    shared = {}
    for g in ("fg", "in", "th", "ot"):
        # x-part first, then A-part (matches kernel chunk order)
        merged = np.concatenate([np.asarray(inputs[f"B_{g}"], f32), Am[g]], axis=0)
        shared[f"W_{g}"] = _chunked_T(merged.astype(f16), KW).reshape(128, KW, MD)
    shared["wball16"] = np.concatenate(
        [np.asarray(inputs[f"b_{g}"], f32).reshape(-1) for g in ("fg", "in", "th", "ot")]
        + [np.ones(128, f32)]
    ).astype(f16).reshape(1, -1)

    ast_c = _chunked_T(np.asarray(inputs["A_st"], f32).astype(f16), KM)
    aptd_c = _chunked_T(
        np.concatenate(
            [A_pt[128 * c : 128 * (c + 1), 128 * c : 128 * (c + 1)] for c in range(16)],
            axis=0,
        ).astype(f16),
        16,
    )
    ident = np.eye(128, dtype=f16)

    in_maps = []
    for i in range(NCORES):
        sl = slice(BL * i, BL * (i + 1))
        xs, ms, as_ = x_t[sl], m_t[sl], a_t[sl]
        im = dict(shared)
        mxT = np.concatenate(
            [np.ascontiguousarray(xs.T), np.ascontiguousarray(ms.T)], axis=0
        ).astype(f16)
        im["mxT16"] = _chunked_T(mxT, KW).reshape(128, KW, 128)
        alast_c = _chunked_T(np.ascontiguousarray(as_[:, 3::4].T).astype(f16), KM)
        at_c = _chunked_T(np.ascontiguousarray(as_.T).astype(f16), 16)
        im["stat_a16"] = np.concatenate([alast_c, at_c, aptd_c], axis=1)
        im["stat_b16"] = np.concatenate([ast_c, as_.astype(f16), ident], axis=1)
        in_maps.append(im)
    return in_maps


def kernel(**inputs):
    global LAST_RESULT
    nc = _get_nc()
    in_maps = _prep_inputs(inputs)
    res = run_bass_kernel_spmd(nc, in_maps, list(range(NCORES)))
    LAST_RESULT = res
    m_new = np.concatenate([res.results[i]["m_new_out"] for i in range(NCORES)], axis=0)
    a_new = np.concatenate([res.results[i]["a_new_out"] for i in range(NCORES)], axis=0)
    return (m_new, a_new)
